# revision 1
# baseline (speedup 1.0000x reference)
"""Trainium2 Bass kernel for nn_BivariateNormalAttention.

Self-contained: takes FULL inputs (B=16), shards batch across 8 NeuronCores
(2 images/core), runs a Bass/Tile kernel per core, gathers [16,8,56,56].

Pipeline per image:
  conv3x3(512->256)+BN+ReLU -> conv3x3(256->256)+BN+ReLU -> avgpool16 (7x7)
  -> conv3x3(256->128)+BN+ReLU -> conv3x3(128->128)+BN+ReLU -> avgpool3s2 (3x3)
  -> conv3x3(128->64)+BN+ReLU -> fc(576->128) -> bivariate-normal attention maps.

Convs 1-2 (99.9% of FLOPs) run as 9-tap shifted matmuls in fp32r (TF32-like,
full PE rate at N>=256). Convs 3-5 / fc / attention run in fp32.
BN scales, pool normalizers and mix-weight softmax are folded on the host.
"""
import sys
import numpy as np

for _p in ("/opt/trn_rl_repo", "/root/.axon_site/_ro/trn_rl_repo"):
    if _p not in sys.path:
        sys.path.append(_p)

import concourse.bacc as bacc
import concourse.mybir as mybir
import concourse.tile as tile
from concourse.bass_utils import run_bass_kernel_spmd

F32 = mybir.dt.float32

B, C, H, W = 16, 512, 112, 112
OUT, GMM = 8, 4
NCORE = 8
IMG = B // NCORE                 # 2 images per core
HP, WP = H + 2, W + 2            # 114
FLAT = HP * WP                   # 12996
RS = 4                           # conv strip rows
NSTRIP = H // RS                 # 28
NFREE = RS * WP                  # 456
H2 = W2 = H // 2                 # 56
SIG2 = float(H) / 2.0            # sigma = 56
LOGR = float(np.log(3.0))


def _conv_dt(conv_dtype):
    return {"f32r": mybir.dt.float32r, "bf16": mybir.dt.bfloat16,
            "f32": mybir.dt.float32}[conv_dtype]


def build_nc(conv_dtype="f32r", r_loop=None, ldma="gpsimd", xbufs=3, pbufs=4,
             cbufs=1, phases="abc"):
    """Bass program for one core processing IMG images."""
    CDT = _conv_dt(conv_dtype)
    nc = bacc.Bacc("TRN2", target_bir_lowering=False, debug=False)

    x = nc.dram_tensor("x", [IMG, 4, 128, HP, WP], CDT, kind="ExternalInput")
    w1t = nc.dram_tensor("w1t", [128, 9, 4, 256], CDT, kind="ExternalInput")
    w2t = nc.dram_tensor("w2t", [128, 9, 2, 256], CDT, kind="ExternalInput")
    w3t = nc.dram_tensor("w3t", [128, 9, 2, 128], F32, kind="ExternalInput")
    w4t = nc.dram_tensor("w4t", [128, 9, 128], F32, kind="ExternalInput")
    w5t = nc.dram_tensor("w5t", [128, 9, 64], F32, kind="ExternalInput")
    wfct = nc.dram_tensor("wfct", [64, 9, 128], F32, kind="ExternalInput")
    b1d = nc.dram_tensor("b1d", [128, 2], F32, kind="ExternalInput")
    b2d = nc.dram_tensor("b2d", [128, 2], F32, kind="ExternalInput")
    b3d = nc.dram_tensor("b3d", [128, 1], F32, kind="ExternalInput")
    b4d = nc.dram_tensor("b4d", [128, 1], F32, kind="ExternalInput")
    b5d = nc.dram_tensor("b5d", [64, 1], F32, kind="ExternalInput")
    selpd = nc.dram_tensor("selpd", [128, 128], F32, kind="ExternalInput")
    selgd = nc.dram_tensor("selgd", [32, 8], F32, kind="ExternalInput")
    negiod = nc.dram_tensor("negiod", [32, 56], F32, kind="ExternalInput")
    cstd = nc.dram_tensor("cstd", [32, 1], F32, kind="ExternalInput")  # -ln(3)

    c1p = nc.dram_tensor("c1p", [IMG, 2, 128, HP, WP], CDT)
    out = nc.dram_tensor("out", [IMG, OUT, H2, W2], F32, kind="ExternalOutput")

    ldma_eng = {"gpsimd": nc.gpsimd, "sync": nc.sync}[ldma]

    with tile.TileContext(nc) as tc:
        def emit_prologue():
            # zero the static borders of c1p once per kernel call
            with tc.tile_pool(name="zpool", bufs=1) as zp:
                if conv_dtype == "f32r":
                    ztf = zp.tile([128, WP], F32)
                    nc.gpsimd.memset(ztf[:], 0.0)
                    zt = ztf[:].bitcast(mybir.dt.float32r)
                else:
                    ztt = zp.tile([128, WP], CDT)
                    nc.gpsimd.memset(ztt[:], 0.0)
                    zt = ztt[:]
                for img in range(IMG):
                    for co in range(2):
                        for row in (0, HP - 1):
                            nc.gpsimd.dma_start(c1p[img, co, :, row, :], zt[:])

        def emit_body():
            # ---------------- persistent small tiles ----------------
            with (
                tc.tile_pool(name="persist", bufs=1) as pp,
            ):
                b1 = pp.tile([128, 2], F32)
                b2 = pp.tile([128, 2], F32)
                nc.gpsimd.dma_start(b1[:], b1d[:])
                nc.gpsimd.dma_start(b2[:], b2d[:])
                # pool accumulators [img][chunk] -> [128, 49]
                pacc = [[pp.tile([128, 49], F32, name=f"pacc{i}_{c}",
                                 tag=f"pacc{i}_{c}")
                         for c in range(2)] for i in range(IMG)]

                # ---------------- phase A: conv1 ----------------
                with (
                    tc.tile_pool(name="w1p", bufs=1) as w1p,
                    tc.tile_pool(name="xa", bufs=xbufs) as xa,
                    tc.tile_pool(name="oa", bufs=3) as oa,
                    tc.tile_pool(name="psa", bufs=pbufs, space="PSUM") as psa,
                ):
                    w1 = w1p.tile([128, 9, 4, 256], CDT)
                    nc.gpsimd.dma_start(w1[:], w1t[:])
                    for img in (range(IMG) if "a" in phases else []):
                        xflat = x[img].rearrange("c p a b -> c p (a b)")
                        for s in range(NSTRIP):
                            nload = min(6 * WP + 2, FLAT - RS * s * WP)
                            xts = []
                            for ci in range(4):
                                xt = xa.tile([128, 6 * WP + 2], CDT, tag=f"x{ci}")
                                ldma_eng.dma_start(
                                    xt[:, :nload],
                                    xflat[ci, :, RS * s * WP:RS * s * WP + nload])
                                xts.append(xt)
                            for co in range(2):
                                p = psa.tile([128, NFREE], F32, tag="psa")
                                k = 0
                                for ci in range(4):
                                    for t in range(9):
                                        nc.tensor.matmul(
                                            p[:],
                                            w1[:, t, ci, co * 128:(co + 1) * 128],
                                            xts[ci][:, (t // 3) * WP + t % 3:
                                                    (t // 3) * WP + t % 3 + NFREE],
                                            start=(k == 0), stop=(k == 35))
                                        k += 1
                                ot = oa.tile([128, RS, WP], CDT, tag=f"o{co}")
                                for _bc in (ot[:, :, 0:1], ot[:, :, WP - 1:WP]):
                                    if conv_dtype == "f32r":
                                        _bc = _bc.bitcast(F32)
                                    nc.vector.memset(_bc, 0.0)
                                nc.scalar.activation(
                                    ot[:, :, 1:113],
                                    p[:].rearrange("p (a b) -> p a b", b=WP)[:, :, 0:112],
                                    mybir.ActivationFunctionType.Relu,
                                    bias=b1[:, co:co + 1])
                                nc.gpsimd.dma_start(
                                    c1p[img, co, :, 1 + RS * s:1 + RS * (s + 1), :],
                                    ot[:])

                # ---------------- phase B: conv2 + avgpool16 ----------------
                with (
                    tc.tile_pool(name="w2p", bufs=1) as w2p,
                    tc.tile_pool(name="xb", bufs=xbufs) as xb,
                    tc.tile_pool(name="ob", bufs=3) as ob,
                    tc.tile_pool(name="psb", bufs=pbufs, space="PSUM") as psb,
                ):
                    w2 = w2p.tile([128, 9, 2, 256], CDT)
                    nc.gpsimd.dma_start(w2[:], w2t[:])
                    for img in (range(IMG) if "b" in phases else []):
                        for c in range(2):
                            nc.vector.memset(pacc[img][c][:], 0.0)
                        cflat = c1p[img].rearrange("c p a b -> c p (a b)")
                        for s in range(NSTRIP):
                            nload = min(6 * WP + 2, FLAT - RS * s * WP)
                            cts = []
                            for ci in range(2):
                                ct = xb.tile([128, 6 * WP + 2], CDT, tag=f"c{ci}")
                                ldma_eng.dma_start(
                                    ct[:, :nload],
                                    cflat[ci, :, RS * s * WP:RS * s * WP + nload])
                                cts.append(ct)
                            for co in range(2):
                                p = psb.tile([128, NFREE], F32, tag="psb")
                                k = 0
                                for ci in range(2):
                                    for t in range(9):
                                        nc.tensor.matmul(
                                            p[:],
                                            w2[:, t, ci, co * 128:(co + 1) * 128],
                                            cts[ci][:, (t // 3) * WP + t % 3:
                                                    (t // 3) * WP + t % 3 + NFREE],
                                            start=(k == 0), stop=(k == 17))
                                        k += 1
                                et = ob.tile([128, RS, WP], F32, tag=f"e{co}")
                                nc.scalar.activation(
                                    et[:], p[:].rearrange("p (a b) -> p a b", b=WP),
                                    mybir.ActivationFunctionType.Relu,
                                    bias=b2[:, co:co + 1])
                                # pool: one XY-reduce over [4 rows x 16 cols]
                                rs_ = ob.tile([128, 7], F32, tag=f"rs{co}")
                                nc.vector.reduce_sum(
                                    rs_[:],
                                    et[:, :, 0:112].rearrange(
                                        "p r (g c) -> p g r c", c=16),
                                    axis=mybir.AxisListType.XY)
                                blk = s // 4
                                nc.vector.tensor_add(
                                    pacc[img][co][:, blk * 7:(blk + 1) * 7],
                                    pacc[img][co][:, blk * 7:(blk + 1) * 7],
                                    rs_[:])

                # ---------------- phase C: head ----------------
                with (
                    tc.tile_pool(name="wc", bufs=1) as wc,
                    tc.tile_pool(name="hc", bufs=cbufs) as hc,
                    tc.tile_pool(name="att", bufs=cbufs) as attp,
                    tc.tile_pool(name="psc", bufs=cbufs, space="PSUM") as psc,
                ):
                    w3 = wc.tile([128, 9, 2, 128], F32)
                    w4 = wc.tile([128, 9, 128], F32)
                    w5 = wc.tile([128, 9, 64], F32)
                    wfc = wc.tile([64, 9, 128], F32)
                    b3 = wc.tile([128, 1], F32)
                    b4 = wc.tile([128, 1], F32)
                    b5 = wc.tile([64, 1], F32)
                    selp = wc.tile([128, 128], F32)
                    selg = wc.tile([32, 8], F32)
                    negio = wc.tile([32, 56], F32)
                    cst = wc.tile([32, 1], F32)
                    for tdst, tsrc in ((w3, w3t), (w4, w4t), (w5, w5t),
                                       (wfc, wfct), (b3, b3d), (b4, b4d),
                                       (b5, b5d), (selp, selpd), (selg, selgd),
                                       (negio, negiod), (cst, cstd)):
                        nc.gpsimd.dma_start(tdst[:], tsrc[:])

                    for img in (range(IMG) if "c" in phases else []):
                        # conv3 (7x7, 256->128): padded 9x9 inputs (+2 tail)
                        p3in = []
                        for ci in range(2):
                            pi = hc.tile([128, 83], F32, tag=f"p3in{ci}")
                            nc.vector.memset(pi[:], 0.0)
                            nc.vector.tensor_copy(
                                pi[:, 10:73].rearrange("p (a b) -> p a b", b=9)[:, :, 0:7],
                                pacc[img][ci][:].rearrange("p (a b) -> p a b", b=7))
                            p3in.append(pi)
                        ps3 = psc.tile([128, 63], F32, tag="ps3")
                        k = 0
                        for ci in range(2):
                            for t in range(9):
                                nc.tensor.matmul(
                                    ps3[:], w3[:, t, ci, :],
                                    p3in[ci][:, (t // 3) * 9 + t % 3:
                                             (t // 3) * 9 + t % 3 + 63],
                                    start=(k == 0), stop=(k == 17))
                                k += 1
                        p4in = hc.tile([128, 83], F32, tag="p4in")
                        nc.vector.memset(p4in[:], 0.0)
                        nc.scalar.activation(
                            p4in[:, 10:73].rearrange("p (a b) -> p a b", b=9)[:, :, 0:7],
                            ps3[:].rearrange("p (a b) -> p a b", b=9)[:, :, 0:7],
                            mybir.ActivationFunctionType.Relu, bias=b3[:, 0:1])
                        # conv4 (7x7, 128->128)
                        ps4 = psc.tile([128, 63], F32, tag="ps4")
                        for t in range(9):
                            nc.tensor.matmul(
                                ps4[:], w4[:, t, :],
                                p4in[:, (t // 3) * 9 + t % 3:
                                     (t // 3) * 9 + t % 3 + 63],
                                start=(t == 0), stop=(t == 8))
                        c4t = hc.tile([128, 49], F32, tag="c4t")
                        nc.scalar.activation(
                            c4t[:].rearrange("p (a b) -> p a b", b=7),
                            ps4[:].rearrange("p (a b) -> p a b", b=9)[:, :, 0:7],
                            mybir.ActivationFunctionType.Relu, bias=b4[:, 0:1])
                        # avgpool 3x3 stride 2 (sum; /9 folded into w5):
                        # overlapping windows via step-2 slices + adds
                        c4v = c4t[:].rearrange("p (y x) -> p y x", x=7)
                        a1 = hc.tile([128, 7, 3], F32, tag="a1")
                        nc.vector.tensor_add(a1[:], c4v[:, :, 0:5:2],
                                             c4v[:, :, 1:6:2])
                        nc.vector.tensor_add(a1[:], a1[:], c4v[:, :, 2:7:2])
                        a2 = hc.tile([128, 9], F32, tag="a2")
                        a2v = a2[:].rearrange("p (i j) -> p i j", j=3)
                        nc.vector.tensor_add(a2v, a1[:, 0:5:2, :],
                                             a1[:, 1:6:2, :])
                        nc.vector.tensor_add(a2v, a2v, a1[:, 2:7:2, :])
                        # conv5 (3x3, 128->64): padded 5x5 (+2 tail)
                        p5in = hc.tile([128, 27], F32, tag="p5in")
                        nc.vector.memset(p5in[:], 0.0)
                        nc.vector.tensor_copy(
                            p5in[:, 6:21].rearrange("p (a b) -> p a b", b=5)[:, :, 0:3],
                            a2[:].rearrange("p (a b) -> p a b", b=3))
                        ps5 = psc.tile([64, 15], F32, tag="ps5")
                        for t in range(9):
                            nc.tensor.matmul(
                                ps5[:], w5[:, t, :],
                                p5in[:, (t // 3) * 5 + t % 3:
                                     (t // 3) * 5 + t % 3 + 15],
                                start=(t == 0), stop=(t == 8))
                        h5 = hc.tile([64, 9], F32, tag="h5")
                        nc.scalar.activation(
                            h5[:].rearrange("p (a b) -> p a b", b=3),
                            ps5[:].rearrange("p (a b) -> p a b", b=5)[:, :, 0:3],
                            mybir.ActivationFunctionType.Relu, bias=b5[:, 0:1])
                        # fc 576->128 as 9 accumulating matmuls (K=64)
                        psf = psc.tile([128, 1], F32, tag="psf")
                        for t in range(9):
                            nc.tensor.matmul(psf[:], wfc[:, t, :], h5[:, t:t + 1],
                                             start=(t == 0), stop=(t == 8))
                        sig = hc.tile([128, 1], F32, tag="sig")
                        nc.scalar.activation(sig[:], psf[:],
                                             mybir.ActivationFunctionType.Sigmoid)
                        # params: one selector matmul -> [mx | my | t | rho']
                        psl = psc.tile([128, 1], F32, tag="psl")
                        nc.tensor.matmul(psl[:], selp[:], sig[:],
                                         start=True, stop=True)
                        mx = hc.tile([32, 1], F32, tag="mx")
                        my = hc.tile([32, 1], F32, tag="my")
                        nc.vector.tensor_copy(mx[:], psl[0:32])
                        nc.vector.tensor_copy(my[:], psl[32:64])
                        r32 = hc.tile([32, 1], F32, tag="r32")
                        nc.scalar.activation(r32[:], psl[64:96],
                                             mybir.ActivationFunctionType.Exp,
                                             bias=cst[:, 0:1])
                        rho = hc.tile([32, 1], F32, tag="rho")
                        nc.vector.tensor_scalar(rho[:], psl[96:128], -0.8, None,
                                                mybir.AluOpType.add)
                        rr = hc.tile([32, 1], F32, tag="rr")
                        nc.vector.tensor_mul(rr[:], rho[:], rho[:])
                        om = hc.tile([32, 1], F32, tag="om")
                        nc.vector.tensor_scalar(om[:], rr[:], -1.0, 1.0,
                                                mybir.AluOpType.mult,
                                                mybir.AluOpType.add)
                        iom = hc.tile([32, 1], F32, tag="iom")
                        nc.vector.reciprocal(iom[:], om[:])
                        den = hc.tile([32, 1], F32, tag="den")
                        nc.vector.tensor_scalar(den[:], iom[:],
                                                -0.5 / (SIG2 * SIG2), None,
                                                mybir.AluOpType.mult)
                        ai = hc.tile([32, 1], F32, tag="ai")
                        nc.vector.tensor_mul(ai[:], den[:], r32[:])
                        ir = hc.tile([32, 1], F32, tag="ir")
                        nc.vector.reciprocal(ir[:], r32[:])
                        bj = hc.tile([32, 1], F32, tag="bj")
                        nc.vector.tensor_mul(bj[:], den[:], ir[:])
                        cc = hc.tile([32, 1], F32, tag="cc")
                        nc.vector.scalar_tensor_tensor(
                            cc[:], den[:], -2.0, rho[:],
                            mybir.AluOpType.mult, mybir.AluOpType.mult)
                        dx = hc.tile([32, 56], F32, tag="dx")
                        nc.vector.tensor_scalar(dx[:], negio[:], mx[:, 0:1], None,
                                                mybir.AluOpType.add)
                        dy = hc.tile([32, 56], F32, tag="dy")
                        nc.vector.tensor_scalar(dy[:], negio[:], my[:, 0:1], None,
                                                mybir.AluOpType.add)
                        u = hc.tile([32, 56], F32, tag="u")
                        nc.vector.scalar_tensor_tensor(
                            u[:], dx[:], ai[:, 0:1], dx[:],
                            mybir.AluOpType.mult, mybir.AluOpType.mult)
                        v = hc.tile([32, 56], F32, tag="v")
                        nc.vector.scalar_tensor_tensor(
                            v[:], dy[:], bj[:, 0:1], dy[:],
                            mybir.AluOpType.mult, mybir.AluOpType.mult)
                        lt = attp.tile([32, 56, 56], F32, tag="lt")
                        nc.vector.scalar_tensor_tensor(
                            lt[:], dx[:].unsqueeze(2).broadcast_to([32, 56, 56]),
                            cc[:, 0:1],
                            dy[:].unsqueeze(1).broadcast_to([32, 56, 56]),
                            mybir.AluOpType.mult, mybir.AluOpType.mult)
                        nc.vector.tensor_add(
                            lt[:], lt[:],
                            u[:].unsqueeze(2).broadcast_to([32, 56, 56]))
                        nc.vector.tensor_add(
                            lt[:], lt[:],
                            v[:].unsqueeze(1).broadcast_to([32, 56, 56]))
                        att = attp.tile([32, 56 * 56], F32, tag="att")
                        asum = hc.tile([32, 1], F32, tag="asum")
                        nc.scalar.activation(
                            att[:], lt[:].rearrange("p a b -> p (a b)"),
                            mybir.ActivationFunctionType.Exp,
                            accum_out=asum[:])
                        inv = hc.tile([32, 1], F32, tag="inv")
                        nc.vector.reciprocal(inv[:], asum[:])
                        nc.vector.tensor_scalar(att[:], att[:], inv[:, 0:1], None,
                                                mybir.AluOpType.mult)
                        obuf = attp.tile([8, 56 * 56], F32, tag="obuf")
                        for ch in range(7):
                            pso = psc.tile([8, 448], F32, tag="pso")
                            nc.tensor.matmul(pso[:], selg[:],
                                             att[:, ch * 448:(ch + 1) * 448],
                                             start=True, stop=True)
                            nc.vector.tensor_copy(
                                obuf[:, ch * 448:(ch + 1) * 448], pso[:])
                        nc.gpsimd.dma_start(
                            out[img].rearrange("o a b -> o (a b)"), obuf[:])

        emit_prologue()
        if r_loop:
            with tc.For_i(0, r_loop, 1):
                emit_body()
        else:
            emit_body()
    nc.compile()
    return nc


def prep_inputs(inputs, conv_dtype="f32r"):
    """Host prep: fold BN/pool scales, build device layouts, shard batch."""
    x = inputs["x"]
    eps_s = 1.0 / np.sqrt(np.float32(1.0 + 1e-5))

    def fold(w, g):
        s = (g * eps_s).astype(np.float32)
        return (w * s[:, None, None, None]).astype(np.float32)

    w1 = fold(inputs["w1"], inputs["g1"])            # [256,512,3,3]
    w2 = fold(inputs["w2"], inputs["g2"])            # [256,256,3,3]
    w3 = fold(inputs["w3"], inputs["g3"]) / 256.0    # avgpool16 norm
    w4 = fold(inputs["w4"], inputs["g4"])
    w5 = fold(inputs["w5"], inputs["g5"]) / 9.0      # avgpool3 norm
    wfc = np.asarray(inputs["w_fc"], np.float32)     # [128, 576]
    mw = np.asarray(inputs["mix_w"], np.float32).reshape(OUT, GMM)
    mw = np.exp(mw - mw.max(1, keepdims=True))
    mw = mw / mw.sum(1, keepdims=True)               # softmax over gmm

    # conv weights -> [128(p=cin%128), 9(tap), ncin, cout]
    def wt_layout(w, ncin):
        co, ci = w.shape[0], w.shape[1]
        r = w.transpose(1, 2, 3, 0).reshape(ncin, 128, 9, co)  # [ncin,128,9,co]
        return np.ascontiguousarray(r.transpose(1, 2, 0, 3))   # [128,9,ncin,co]

    w1t = wt_layout(w1, 4)
    w2t = wt_layout(w2, 2)
    w3t = wt_layout(w3, 2)
    w4t = wt_layout(w4, 1)[:, :, 0, :]
    w5t = wt_layout(w5, 1)[:, :, 0, :]
    # wfct[c, s, o] = wfc[o, c*9+s]
    wfct = np.ascontiguousarray(wfc.reshape(128, 64, 9).transpose(1, 2, 0))

    def bias_chunks(b, nchunk):
        return np.ascontiguousarray(
            np.asarray(b, np.float32).reshape(nchunk, 128).T)

    b1h = bias_chunks(inputs["b1"], 2)
    b2h = bias_chunks(inputs["b2"], 2)
    b3h = np.asarray(inputs["b3"], np.float32).reshape(128, 1)
    b4h = np.asarray(inputs["b4"], np.float32).reshape(128, 1)
    b5h = np.asarray(inputs["b5"], np.float32).reshape(64, 1)

    selp = np.zeros((128, 128), np.float32)
    for m in range(32):
        selp[4 * m + 0, m] = float(H2 - 1)
        selp[4 * m + 1, m + 32] = float(W2 - 1)
        selp[4 * m + 2, m + 64] = 2.0 * LOGR
        selp[4 * m + 3, m + 96] = 1.6
    selg = np.zeros((32, 8), np.float32)
    for o in range(OUT):
        for g in range(GMM):
            selg[o * GMM + g, o] = mw[o, g]
    negio = np.broadcast_to(-np.arange(56, dtype=np.float32), (32, 56)).copy()
    cst = np.full((32, 1), -LOGR, np.float32)

    np_dt = np.float32
    if conv_dtype == "bf16":
        import ml_dtypes
        np_dt = ml_dtypes.bfloat16

    xp = np.zeros((B, 4, 128, HP, WP), np_dt)
    xp[:, :, :, 1:113, 1:113] = np.asarray(x, np.float32).reshape(
        B, 4, 128, H, W).astype(np_dt)

    common = {
        "w1t": w1t.astype(np_dt), "w2t": w2t.astype(np_dt),
        "w3t": w3t, "w4t": w4t, "w5t": w5t, "wfct": wfct,
        "b1d": b1h, "b2d": b2h, "b3d": b3h, "b4d": b4h, "b5d": b5h,
        "selpd": selp, "selgd": selg, "negiod": negio, "cstd": cst,
    }
    in_maps = []
    for c in range(NCORE):
        m = dict(common)
        m["x"] = np.ascontiguousarray(xp[c * IMG:(c + 1) * IMG])
        in_maps.append(m)
    return in_maps


_CACHE = {}


def kernel(**inputs):
    inputs = {k: np.asarray(v) for k, v in inputs.items()}
    conv_dtype = "f32r"
    if "nc" not in _CACHE:
        _CACHE["nc"] = build_nc(conv_dtype)
    nc = _CACHE["nc"]
    in_maps = prep_inputs(inputs, conv_dtype)
    res = run_bass_kernel_spmd(nc, in_maps, core_ids=list(range(NCORE)))
    out = np.concatenate([res.results[c]["out"] for c in range(NCORE)], axis=0)
    return np.ascontiguousarray(out.astype(np.float32))



# revision 4
# speedup vs baseline: 2.0362x; 2.0362x over previous
"""Trainium2 Bass kernel for nn_BivariateNormalAttention.

Self-contained: takes FULL inputs (B=16), shards batch across 8 NeuronCores
(2 images/core), runs a Bass/Tile kernel per core, gathers [16,8,56,56].

Pipeline per image:
  conv3x3(512->256)+BN+ReLU -> conv3x3(256->256)+BN+ReLU -> avgpool16 (7x7)
  -> conv3x3(256->128)+BN+ReLU -> conv3x3(128->128)+BN+ReLU -> avgpool3s2 (3x3)
  -> conv3x3(128->64)+BN+ReLU -> fc(576->128) -> bivariate-normal attention maps.

Convs 1-2 (99.9% of FLOPs) run as 9-tap shifted matmuls in fp8-e4m3 with
perf_mode=DoubleRow (two 128-deep K blocks per instruction at 0.5 cyc/row).
Weight loads amortize over groups of 4 row-strips. The fp8 weight-quantization
bias (which couples to the nonzero mean of post-ReLU activations) is removed
by a per-image bias correction: corr = (sum_taps dw2) @ mean(h1), computed on
device from a running sum of conv1 activations. Convs 3-5 / fc / attention run
in fp32; the attention tail batches both images on 64 partitions.
"""
import sys
import numpy as np

for _p in ("/opt/trn_rl_repo", "/root/.axon_site/_ro/trn_rl_repo"):
    if _p not in sys.path:
        sys.path.append(_p)

import concourse.bacc as bacc
import concourse.mybir as mybir
import concourse.tile as tile
from concourse.bass_utils import run_bass_kernel_spmd

F32 = mybir.dt.float32
F8 = mybir.dt.float8e4
DR = mybir.MatmulPerfMode.DoubleRow

B, C, H, W = 16, 512, 112, 112
OUT, GMM = 8, 4
NCORE = 8
IMG = B // NCORE                 # 2 images per core
HP, WP = H + 2, W + 2            # 114
FLAT = HP * WP                   # 12996
RS = 4                           # conv strip rows
NSTRIP = H // RS                 # 28
GS = 4                           # strips per weight-load group
NGRP = NSTRIP // GS              # 7
NFREE = RS * WP                  # 456
XL = 688                         # per-ci lane length (6*WP+2=686 padded to 16)
H2 = W2 = H // 2                 # 56
SIG2 = float(H) / 2.0            # sigma = 56
LOGR = float(np.log(3.0))


def build_nc(conv_dtype="fp8", r_loop=None, ldma="sync", xbufs=8, pbufs=8,
             cbufs=1, phases="abc"):
    """Bass program for one core processing IMG images."""
    nc = bacc.Bacc("TRN2", target_bir_lowering=False, debug=False)

    x = nc.dram_tensor("x", [IMG, 4, 128, HP, WP], F8, kind="ExternalInput")
    w1t = nc.dram_tensor("w1t", [128, 9, 4, 256], F8, kind="ExternalInput")
    w2t = nc.dram_tensor("w2t", [128, 9, 2, 256], F8, kind="ExternalInput")
    w3t = nc.dram_tensor("w3t", [128, 9, 2, 128], F32, kind="ExternalInput")
    w4t = nc.dram_tensor("w4t", [128, 9, 128], F32, kind="ExternalInput")
    w5t = nc.dram_tensor("w5t", [128, 9, 64], F32, kind="ExternalInput")
    wfct = nc.dram_tensor("wfct", [64, 9, 128], F32, kind="ExternalInput")
    b1d = nc.dram_tensor("b1d", [128, 2], F32, kind="ExternalInput")
    b2d = nc.dram_tensor("b2d", [128, 2], F32, kind="ExternalInput")
    b3d = nc.dram_tensor("b3d", [128, 1], F32, kind="ExternalInput")
    b4d = nc.dram_tensor("b4d", [128, 1], F32, kind="ExternalInput")
    b5d = nc.dram_tensor("b5d", [64, 1], F32, kind="ExternalInput")
    selpd = nc.dram_tensor("selpd", [128, 128], F32, kind="ExternalInput")
    selgd = nc.dram_tensor("selgd", [64, 16], F32, kind="ExternalInput")
    negiod = nc.dram_tensor("negiod", [64, 56], F32, kind="ExternalInput")
    cstd = nc.dram_tensor("cstd", [64, 1], F32, kind="ExternalInput")  # -ln3
    selSd = nc.dram_tensor("selSd", [128, 2, 2, 128], F32,
                           kind="ExternalInput")  # conv2 fp8 mean correction

    c1p = nc.dram_tensor("c1p", [IMG, 2, 128, HP, WP], F8)
    out = nc.dram_tensor("out", [IMG, OUT, H2, W2], F32, kind="ExternalOutput")

    ldma_eng = {"gpsimd": nc.gpsimd, "sync": nc.sync}[ldma]

    with tile.TileContext(nc) as tc:
        def emit_prologue():
            # zero the static borders of c1p once per kernel call
            with tc.tile_pool(name="zpool", bufs=1) as zp:
                zt = zp.tile([128, WP], F8)
                nc.gpsimd.memset(zt[:], 0.0)
                for img in range(IMG):
                    for co in range(2):
                        for row in (0, HP - 1):
                            nc.gpsimd.dma_start(c1p[img, co, :, row, :], zt[:])
                        for col in (0, WP - 1):
                            nc.gpsimd.dma_start(
                                c1p[img, co, :, 1:HP - 1, col],
                                zt[:, 0:HP - 2])

        def emit_body():
            with (
                tc.tile_pool(name="persist", bufs=1) as pp,
            ):
                b1 = pp.tile([128, 2], F32)
                b2 = pp.tile([128, 2], F32)
                selS = pp.tile([128, 2, 2, 128], F32)
                nc.gpsimd.dma_start(b1[:], b1d[:])
                nc.gpsimd.dma_start(b2[:], b2d[:])
                nc.gpsimd.dma_start(selS[:], selSd[:])
                # per-(img, chunk) sums of conv1 activations / pool accums
                hsum = [[pp.tile([128, 1], F32, name=f"hs{i}_{c}",
                                 tag=f"hs{i}_{c}")
                         for c in range(2)] for i in range(IMG)]
                b2eff = [pp.tile([128, 2], F32, name=f"b2e{i}",
                                 tag=f"b2e{i}")
                         for i in range(IMG)]
                pacc = [[pp.tile([128, 49], F32, name=f"pacc{i}_{c}",
                                 tag=f"pacc{i}_{c}")
                         for c in range(2)] for i in range(IMG)]

                # ---------------- phase A: conv1 (fp8 DoubleRow) ------------
                with (
                    tc.tile_pool(name="w1p", bufs=1) as w1p,
                    tc.tile_pool(name="xa", bufs=xbufs) as xa,
                    tc.tile_pool(name="oa", bufs=6) as oa,
                    tc.tile_pool(name="psa", bufs=pbufs, space="PSUM") as psa,
                ):
                    w1 = w1p.tile([128, 9, 4, 256], F8)
                    nc.gpsimd.dma_start(w1[:], w1t[:])
                    for img in (range(IMG) if "a" in phases else []):
                        for c in range(2):
                            nc.vector.memset(hsum[img][c][:], 0.0)
                        xflat = x[img].rearrange("c p a b -> p c (a b)")
                        for g in range(NGRP):
                            xts = []
                            for s in range(GS):
                                sg = g * GS + s
                                nload = min(6 * WP + 2, FLAT - RS * sg * WP)
                                xt = xa.tile([128, 4, XL], F8, tag=f"x{s}")
                                ldma_eng.dma_start(
                                    xt[:, :, :nload],
                                    xflat[:, :, RS * sg * WP:
                                          RS * sg * WP + nload])
                                xts.append(xt)
                            for co in range(2):
                                ps = [psa.tile([128, NFREE], F32,
                                               name="psa", tag="psa")
                                      for _ in range(GS)]
                                for k in range(18):
                                    p, t = divmod(k, 9)
                                    wap = w1[:, t, 2 * p:2 * p + 2,
                                             co * 128:(co + 1) * 128]
                                    sh = (t // 3) * WP + t % 3
                                    for s in range(GS):
                                        nc.tensor.matmul(
                                            ps[s][:], wap,
                                            xts[s][:, 2 * p:2 * p + 2,
                                                   sh:sh + NFREE],
                                            start=(k == 0), stop=(k == 17),
                                            perf_mode=DR)
                                for s in range(GS):
                                    sg = g * GS + s
                                    ot = oa.tile([128, RS, 112], F8, tag="ot")
                                    rs_ = oa.tile([128, 1], F32, tag="rs")
                                    nc.scalar.activation(
                                        ot[:],
                                        ps[s][:].rearrange(
                                            "p (a b) -> p a b",
                                            b=WP)[:, :, 0:112],
                                        mybir.ActivationFunctionType.Relu,
                                        bias=b1[:, co:co + 1],
                                        accum_out=rs_[:])
                                    nc.gpsimd.dma_start(
                                        c1p[img, co, :,
                                            1 + RS * sg:1 + RS * (sg + 1),
                                            1:113],
                                        ot[:])
                                    nc.vector.tensor_add(
                                        hsum[img][co][:], hsum[img][co][:],
                                        rs_[:])

                # ---------------- conv2 bias correction ---------------------
                with tc.tile_pool(name="pb2", bufs=IMG * 2,
                                  space="PSUM") as pb2:
                    for img in (range(IMG) if "b" in phases else []):
                        for c in range(2):
                            pcc = pb2.tile([128, 1], F32, tag="pcc")
                            nc.tensor.matmul(pcc[:], selS[:, 0, c, :],
                                             hsum[img][0][:],
                                             start=True, stop=False)
                            nc.tensor.matmul(pcc[:], selS[:, 1, c, :],
                                             hsum[img][1][:],
                                             start=False, stop=True)
                            nc.vector.tensor_add(
                                b2eff[img][:, c:c + 1], pcc[:],
                                b2[:, c:c + 1])

                # ---------------- phase B: conv2 + avgpool16 ----------------
                with (
                    tc.tile_pool(name="w2p", bufs=1) as w2p,
                    tc.tile_pool(name="xb", bufs=xbufs) as xb,
                    tc.tile_pool(name="ob", bufs=6) as ob,
                    tc.tile_pool(name="psb", bufs=pbufs, space="PSUM") as psb,
                ):
                    w2 = w2p.tile([128, 9, 2, 256], F8)
                    nc.gpsimd.dma_start(w2[:], w2t[:])
                    for img in (range(IMG) if "b" in phases else []):
                        for c in range(2):
                            nc.vector.memset(pacc[img][c][:], 0.0)
                        cflat = c1p[img].rearrange("c p a b -> p c (a b)")
                        for g in range(NGRP):
                            cts = []
                            for s in range(GS):
                                sg = g * GS + s
                                nload = min(6 * WP + 2, FLAT - RS * sg * WP)
                                ct = xb.tile([128, 2, XL], F8, tag=f"c{s}")
                                ldma_eng.dma_start(
                                    ct[:, :, :nload],
                                    cflat[:, :, RS * sg * WP:
                                          RS * sg * WP + nload])
                                cts.append(ct)
                            for co in range(2):
                                ps = [psb.tile([128, NFREE], F32,
                                               name="psb", tag="psb")
                                      for _ in range(GS)]
                                for t in range(9):
                                    wap = w2[:, t, :,
                                             co * 128:(co + 1) * 128]
                                    sh = (t // 3) * WP + t % 3
                                    for s in range(GS):
                                        nc.tensor.matmul(
                                            ps[s][:], wap,
                                            cts[s][:, :, sh:sh + NFREE],
                                            start=(t == 0), stop=(t == 8),
                                            perf_mode=DR)
                                for s in range(GS):
                                    sg = g * GS + s
                                    et = ob.tile([128, RS, 112], F32,
                                                 tag="et")
                                    nc.scalar.activation(
                                        et[:],
                                        ps[s][:].rearrange(
                                            "p (a b) -> p a b",
                                            b=WP)[:, :, 0:112],
                                        mybir.ActivationFunctionType.Relu,
                                        bias=b2eff[img][:, co:co + 1])
                                    # pool: XY-reduce over [4 rows x 16 cols]
                                    rp = ob.tile([128, 7], F32, tag="rp")
                                    nc.vector.reduce_sum(
                                        rp[:],
                                        et[:].rearrange(
                                            "p r (g c) -> p g r c", c=16),
                                        axis=mybir.AxisListType.XY)
                                    blk = sg // 4
                                    nc.vector.tensor_add(
                                        pacc[img][co][:, blk * 7:
                                                      (blk + 1) * 7],
                                        pacc[img][co][:, blk * 7:
                                                      (blk + 1) * 7],
                                        rp[:])

                # ---------------- phase C: head ----------------
                with (
                    tc.tile_pool(name="wc", bufs=1) as wc,
                    tc.tile_pool(name="hc", bufs=cbufs) as hc,
                    tc.tile_pool(name="att", bufs=cbufs) as attp,
                    tc.tile_pool(name="psc", bufs=cbufs, space="PSUM") as psc,
                ):
                    w3 = wc.tile([128, 9, 2, 128], F32)
                    w4 = wc.tile([128, 9, 128], F32)
                    w5 = wc.tile([128, 9, 64], F32)
                    wfc = wc.tile([64, 9, 128], F32)
                    b3 = wc.tile([128, 1], F32)
                    b4 = wc.tile([128, 1], F32)
                    b5 = wc.tile([64, 1], F32)
                    selp = wc.tile([128, 128], F32)
                    selg = wc.tile([64, 16], F32)
                    negio = wc.tile([64, 56], F32)
                    cst = wc.tile([64, 1], F32)
                    for tdst, tsrc in ((w3, w3t), (w4, w4t), (w5, w5t),
                                       (wfc, wfct), (b3, b3d), (b4, b4d),
                                       (b5, b5d), (selp, selpd), (selg, selgd),
                                       (negio, negiod), (cst, cstd)):
                        nc.gpsimd.dma_start(tdst[:], tsrc[:])

                    # stacked per-(img) attention params [64, 1]
                    mxs = hc.tile([64, 1], F32, tag="mxs")
                    mys = hc.tile([64, 1], F32, tag="mys")
                    tts = hc.tile([64, 1], F32, tag="tts")
                    rhs_ = hc.tile([64, 1], F32, tag="rhs")

                    for img in (range(IMG) if "c" in phases else []):
                        # conv3 (7x7, 256->128): padded 9x9 inputs (+2 tail)
                        p3in = []
                        for ci in range(2):
                            pi = hc.tile([128, 83], F32, tag=f"p3in{ci}")
                            nc.vector.memset(pi[:], 0.0)
                            nc.vector.tensor_copy(
                                pi[:, 10:73].rearrange(
                                    "p (a b) -> p a b", b=9)[:, :, 0:7],
                                pacc[img][ci][:].rearrange(
                                    "p (a b) -> p a b", b=7))
                            p3in.append(pi)
                        ps3 = psc.tile([128, 63], F32, tag="ps3")
                        k = 0
                        for ci in range(2):
                            for t in range(9):
                                nc.tensor.matmul(
                                    ps3[:], w3[:, t, ci, :],
                                    p3in[ci][:, (t // 3) * 9 + t % 3:
                                             (t // 3) * 9 + t % 3 + 63],
                                    start=(k == 0), stop=(k == 17))
                                k += 1
                        p4in = hc.tile([128, 83], F32, tag="p4in")
                        nc.vector.memset(p4in[:], 0.0)
                        nc.scalar.activation(
                            p4in[:, 10:73].rearrange(
                                "p (a b) -> p a b", b=9)[:, :, 0:7],
                            ps3[:].rearrange("p (a b) -> p a b",
                                             b=9)[:, :, 0:7],
                            mybir.ActivationFunctionType.Relu, bias=b3[:, 0:1])
                        # conv4 (7x7, 128->128)
                        ps4 = psc.tile([128, 63], F32, tag="ps4")
                        for t in range(9):
                            nc.tensor.matmul(
                                ps4[:], w4[:, t, :],
                                p4in[:, (t // 3) * 9 + t % 3:
                                     (t // 3) * 9 + t % 3 + 63],
                                start=(t == 0), stop=(t == 8))
                        c4t = hc.tile([128, 49], F32, tag="c4t")
                        nc.scalar.activation(
                            c4t[:].rearrange("p (a b) -> p a b", b=7),
                            ps4[:].rearrange("p (a b) -> p a b",
                                             b=9)[:, :, 0:7],
                            mybir.ActivationFunctionType.Relu, bias=b4[:, 0:1])
                        # avgpool 3x3 stride 2 (sum; /9 folded into w5):
                        c4v = c4t[:].rearrange("p (y x) -> p y x", x=7)
                        a1 = hc.tile([128, 7, 3], F32, tag="a1")
                        nc.vector.tensor_add(a1[:], c4v[:, :, 0:5:2],
                                             c4v[:, :, 1:6:2])
                        nc.vector.tensor_add(a1[:], a1[:], c4v[:, :, 2:7:2])
                        a2 = hc.tile([128, 9], F32, tag="a2")
                        a2v = a2[:].rearrange("p (i j) -> p i j", j=3)
                        nc.vector.tensor_add(a2v, a1[:, 0:5:2, :],
                                             a1[:, 1:6:2, :])
                        nc.vector.tensor_add(a2v, a2v, a1[:, 2:7:2, :])
                        # conv5 (3x3, 128->64): padded 5x5 (+2 tail)
                        p5in = hc.tile([128, 27], F32, tag="p5in")
                        nc.vector.memset(p5in[:], 0.0)
                        nc.vector.tensor_copy(
                            p5in[:, 6:21].rearrange(
                                "p (a b) -> p a b", b=5)[:, :, 0:3],
                            a2[:].rearrange("p (a b) -> p a b", b=3))
                        ps5 = psc.tile([64, 15], F32, tag="ps5")
                        for t in range(9):
                            nc.tensor.matmul(
                                ps5[:], w5[:, t, :],
                                p5in[:, (t // 3) * 5 + t % 3:
                                     (t // 3) * 5 + t % 3 + 15],
                                start=(t == 0), stop=(t == 8))
                        h5 = hc.tile([64, 9], F32, tag="h5")
                        nc.scalar.activation(
                            h5[:].rearrange("p (a b) -> p a b", b=3),
                            ps5[:].rearrange("p (a b) -> p a b",
                                             b=5)[:, :, 0:3],
                            mybir.ActivationFunctionType.Relu, bias=b5[:, 0:1])
                        # fc 576->128 as 9 accumulating matmuls (K=64)
                        psf = psc.tile([128, 1], F32, tag="psf")
                        for t in range(9):
                            nc.tensor.matmul(psf[:], wfc[:, t, :],
                                             h5[:, t:t + 1],
                                             start=(t == 0), stop=(t == 8))
                        sig = hc.tile([128, 1], F32, tag="sig")
                        nc.scalar.activation(sig[:], psf[:],
                                             mybir.ActivationFunctionType.
                                             Sigmoid)
                        # params: one selector matmul -> [mx | my | t | rho']
                        psl = psc.tile([128, 1], F32, tag="psl")
                        nc.tensor.matmul(psl[:], selp[:], sig[:],
                                         start=True, stop=True)
                        o = 32 * img
                        nc.vector.tensor_copy(mxs[o:o + 32], psl[0:32])
                        nc.vector.tensor_copy(mys[o:o + 32], psl[32:64])
                        nc.vector.tensor_copy(tts[o:o + 32], psl[64:96])
                        nc.vector.tensor_copy(rhs_[o:o + 32], psl[96:128])

                    # ---- batched attention for both images on 64 partitions
                    if "c" in phases:
                        r64 = hc.tile([64, 1], F32, tag="r64")
                        nc.scalar.activation(r64[:], tts[:],
                                             mybir.ActivationFunctionType.Exp,
                                             bias=cst[:, 0:1])
                        rho = hc.tile([64, 1], F32, tag="rho")
                        nc.vector.tensor_scalar(rho[:], rhs_[:], -0.8, None,
                                                mybir.AluOpType.add)
                        rr = hc.tile([64, 1], F32, tag="rr")
                        nc.vector.tensor_mul(rr[:], rho[:], rho[:])
                        om = hc.tile([64, 1], F32, tag="om")
                        nc.vector.tensor_scalar(om[:], rr[:], -1.0, 1.0,
                                                mybir.AluOpType.mult,
                                                mybir.AluOpType.add)
                        iom = hc.tile([64, 1], F32, tag="iom")
                        nc.vector.reciprocal(iom[:], om[:])
                        den = hc.tile([64, 1], F32, tag="den")
                        nc.vector.tensor_scalar(den[:], iom[:],
                                                -0.5 / (SIG2 * SIG2), None,
                                                mybir.AluOpType.mult)
                        ai = hc.tile([64, 1], F32, tag="ai")
                        nc.vector.tensor_mul(ai[:], den[:], r64[:])
                        ir = hc.tile([64, 1], F32, tag="ir")
                        nc.vector.reciprocal(ir[:], r64[:])
                        bj = hc.tile([64, 1], F32, tag="bj")
                        nc.vector.tensor_mul(bj[:], den[:], ir[:])
                        cc = hc.tile([64, 1], F32, tag="cc")
                        nc.vector.scalar_tensor_tensor(
                            cc[:], den[:], -2.0, rho[:],
                            mybir.AluOpType.mult, mybir.AluOpType.mult)
                        dx = hc.tile([64, 56], F32, tag="dx")
                        nc.vector.tensor_scalar(dx[:], negio[:], mxs[:, 0:1],
                                                None, mybir.AluOpType.add)
                        dy = hc.tile([64, 56], F32, tag="dy")
                        nc.vector.tensor_scalar(dy[:], negio[:], mys[:, 0:1],
                                                None, mybir.AluOpType.add)
                        u = hc.tile([64, 56], F32, tag="u")
                        nc.vector.scalar_tensor_tensor(
                            u[:], dx[:], ai[:, 0:1], dx[:],
                            mybir.AluOpType.mult, mybir.AluOpType.mult)
                        v = hc.tile([64, 56], F32, tag="v")
                        nc.vector.scalar_tensor_tensor(
                            v[:], dy[:], bj[:, 0:1], dy[:],
                            mybir.AluOpType.mult, mybir.AluOpType.mult)
                        lt = attp.tile([64, 56, 56], F32, tag="lt")
                        nc.vector.scalar_tensor_tensor(
                            lt[:],
                            dx[:].unsqueeze(2).broadcast_to([64, 56, 56]),
                            cc[:, 0:1],
                            dy[:].unsqueeze(1).broadcast_to([64, 56, 56]),
                            mybir.AluOpType.mult, mybir.AluOpType.mult)
                        nc.vector.tensor_add(
                            lt[:], lt[:],
                            u[:].unsqueeze(2).broadcast_to([64, 56, 56]))
                        nc.vector.tensor_add(
                            lt[:], lt[:],
                            v[:].unsqueeze(1).broadcast_to([64, 56, 56]))
                        att = attp.tile([64, 56 * 56], F32, tag="att")
                        asum = hc.tile([64, 1], F32, tag="asum")
                        nc.scalar.activation(
                            att[:], lt[:].rearrange("p a b -> p (a b)"),
                            mybir.ActivationFunctionType.Exp,
                            accum_out=asum[:])
                        inv = hc.tile([64, 1], F32, tag="inv")
                        nc.vector.reciprocal(inv[:], asum[:])
                        # fold row normalization into the mixture selector
                        sg2 = hc.tile([64, 16], F32, tag="sg2")
                        nc.vector.tensor_scalar(sg2[:], selg[:], inv[:, 0:1],
                                                None, mybir.AluOpType.mult)
                        obuf = attp.tile([16, 56 * 56], F32, tag="obuf")
                        for ch in range(7):
                            pso = psc.tile([16, 448], F32, tag="pso")
                            nc.tensor.matmul(pso[:], sg2[:],
                                             att[:, ch * 448:(ch + 1) * 448],
                                             start=True, stop=True)
                            nc.vector.tensor_copy(
                                obuf[:, ch * 448:(ch + 1) * 448], pso[:])
                        nc.gpsimd.dma_start(
                            out.rearrange("i o a b -> (i o) (a b)"), obuf[:])

        emit_prologue()
        if r_loop:
            with tc.For_i(0, r_loop, 1):
                emit_body()
        else:
            emit_body()
    nc.compile()
    return nc


def prep_inputs(inputs, conv_dtype="fp8"):
    """Host prep: fold BN/pool scales, quantize, build layouts, shard batch."""
    import ml_dtypes
    F8NP = ml_dtypes.float8_e4m3

    x = inputs["x"]
    eps_s = 1.0 / np.sqrt(np.float32(1.0 + 1e-5))

    def fold(w, g):
        s = (g * eps_s).astype(np.float32)
        return (w * s[:, None, None, None]).astype(np.float32)

    w1 = fold(inputs["w1"], inputs["g1"])            # [256,512,3,3]
    w2 = fold(inputs["w2"], inputs["g2"])            # [256,256,3,3]
    w3 = fold(inputs["w3"], inputs["g3"]) / 256.0    # avgpool16 norm
    w4 = fold(inputs["w4"], inputs["g4"])
    w5 = fold(inputs["w5"], inputs["g5"]) / 9.0      # avgpool3 norm
    wfc = np.asarray(inputs["w_fc"], np.float32)     # [128, 576]
    mw = np.asarray(inputs["mix_w"], np.float32).reshape(OUT, GMM)
    mw = np.exp(mw - mw.max(1, keepdims=True))
    mw = mw / mw.sum(1, keepdims=True)               # softmax over gmm

    w2q = w2.astype(F8NP).astype(np.float32)
    # conv2 fp8 mean-correction: corr[co] = S @ mean(h1), S = sum_taps dw2
    S = (w2 - w2q).sum(axis=(2, 3)) / float(H * W)   # [co, ci]
    selS = np.ascontiguousarray(
        S.reshape(2, 128, 2, 128).transpose(3, 2, 0, 1))  # [cip,cic,coc,cop]

    # conv weights -> [128(p=cin%128), 9(tap), ncin, cout]
    def wt_layout(w, ncin):
        co = w.shape[0]
        r = w.transpose(1, 2, 3, 0).reshape(ncin, 128, 9, co)
        return np.ascontiguousarray(r.transpose(1, 2, 0, 3))

    w1t = wt_layout(w1.astype(F8NP).astype(np.float32), 4).astype(F8NP)
    w2t = wt_layout(w2q, 2).astype(F8NP)
    w3t = wt_layout(w3, 2)
    w4t = wt_layout(w4, 1)[:, :, 0, :]
    w5t = wt_layout(w5, 1)[:, :, 0, :]
    wfct = np.ascontiguousarray(wfc.reshape(128, 64, 9).transpose(1, 2, 0))

    def bias_chunks(b, nchunk):
        return np.ascontiguousarray(
            np.asarray(b, np.float32).reshape(nchunk, 128).T)

    b1h = bias_chunks(inputs["b1"], 2)
    b2h = bias_chunks(inputs["b2"], 2)
    b3h = np.asarray(inputs["b3"], np.float32).reshape(128, 1)
    b4h = np.asarray(inputs["b4"], np.float32).reshape(128, 1)
    b5h = np.asarray(inputs["b5"], np.float32).reshape(64, 1)

    selp = np.zeros((128, 128), np.float32)
    for m in range(32):
        selp[4 * m + 0, m] = float(H2 - 1)
        selp[4 * m + 1, m + 32] = float(W2 - 1)
        selp[4 * m + 2, m + 64] = 2.0 * LOGR
        selp[4 * m + 3, m + 96] = 1.6
    selg = np.zeros((64, 16), np.float32)
    for i in range(IMG):
        for o in range(OUT):
            for g in range(GMM):
                selg[32 * i + o * GMM + g, 8 * i + o] = mw[o, g]
    negio = np.broadcast_to(-np.arange(56, dtype=np.float32), (64, 56)).copy()
    cst = np.full((64, 1), -LOGR, np.float32)

    xp = np.zeros((B, 4, 128, HP, WP), F8NP)
    xp[:, :, :, 1:113, 1:113] = np.asarray(x, np.float32).reshape(
        B, 4, 128, H, W).astype(F8NP)

    common = {
        "w1t": w1t, "w2t": w2t,
        "w3t": w3t, "w4t": w4t, "w5t": w5t, "wfct": wfct,
        "b1d": b1h, "b2d": b2h, "b3d": b3h, "b4d": b4h, "b5d": b5h,
        "selpd": selp, "selgd": selg, "negiod": negio, "cstd": cst,
        "selSd": selS,
    }
    in_maps = []
    for c in range(NCORE):
        m = dict(common)
        m["x"] = np.ascontiguousarray(xp[c * IMG:(c + 1) * IMG])
        in_maps.append(m)
    return in_maps


_CACHE = {}


def kernel(**inputs):
    inputs = {k: np.asarray(v) for k, v in inputs.items()}
    if "nc" not in _CACHE:
        _CACHE["nc"] = build_nc()
    nc = _CACHE["nc"]
    in_maps = prep_inputs(inputs)
    res = run_bass_kernel_spmd(nc, in_maps, core_ids=list(range(NCORE)))
    out = np.concatenate([res.results[c]["out"] for c in range(NCORE)], axis=0)
    return np.ascontiguousarray(out.astype(np.float32))
